# revision 24
# baseline (speedup 1.0000x reference)
"""Trainium2 Bass kernel for nn_DetModel_77738908057822.

Computation (per batch item b, `iterations` steps):
    HtH_b = H_b^T H_b   (64x64), Hty_b = H_b^T y_b  (64)
    z = x + 2*ss*(Hty - HtH x) = A x + c,  A = I - 2ss*HtH, c = 2ss*Hty
    h' = relu(z @ W_ih^T + h @ W_hh^T + b_ih + b_hh)
    x' = h' @ w_x^T + b_x
Returns (x_N [B,64,1], h_N [B,512]).

Sharding: pure data parallel over batch across 8 cores (512 items/core).

Per-core plan:
  Phase 1 (PE): per-item gram matmuls lhsT=H_b rhs=[H_b | y_b] -> PSUM
     [64, 65] (j on partitions).  Scale by -2ss while copying PSUM->SBUF
     (bf16 out), DMA to DRAM scratch laid out [item, j, 66] (col 65 pad).
  Phase 2: DMA scratch back contiguous -> A [128 items-on-partitions,
     4 groups, 64, 66] bf16; add 1.0 on the diagonal.  Col 64 = -c.
  Phase 3 per iteration:
     PE:  hp[m] += sum_k W_hhT_km^T hT_k      (fp32r, runs during DVE work)
     DVE: per group: t = A * x_bcast (bf16); tree-fold over k;
          z = fold - (-c)  (fp32 out)
     PE:  zT = transpose(z); ACT copy -> zT_sb; hp[m] += W_ihT_m^T zT (fp32r)
     ACT: hT'[m] = relu(hp[m] + bh[m])
     PE:  xT = sum_k wxT_k^T hT'_k (fp32r) -> ACT Identity + b_x -> xT_sb
     PE:  xb[g] = transpose(xT_g)  (PSUM, items on partitions, incl b_x)
     DVE: x_bf = bf16(xb)
  Final x = xb (has b_x); h via 16 PE transposes.
"""

import sys
import numpy as np

for _p in ("/opt/trn_rl_repo",):
    if _p not in sys.path:
        sys.path.insert(0, _p)

B, RX, TX, HID = 4096, 128, 64, 512
NCORES = 8
BL = B // NCORES          # 512 items per core
NG = BL // 128            # 4 groups of 128 items
KCH = TX + 2              # 66: [HtH(64) | Hty(1) | pad(1)] per item row
CH = 64                   # items per staged scratch write
SLOT = 4                  # items per gram psum tile

USE_F32R = True           # fp32r (1 cyc/col) for the big matmuls
USE_BF16_MV = True        # bf16 A + bf16 tree-fold matvec

_BUILD_CACHE = {}


def _build(ss: float, iterations: int):
    from contextlib import ExitStack

    import concourse.bacc as bacc
    import concourse.bass as bass
    import concourse.tile as tile
    from concourse import mybir

    f32 = mybir.dt.float32
    f32r = mybir.dt.float32r
    mvdt = mybir.dt.float16 if USE_BF16_MV else f32
    Relu = mybir.ActivationFunctionType.Relu
    Copy = mybir.ActivationFunctionType.Copy
    Ident = mybir.ActivationFunctionType.Identity
    AXX = mybir.AxisListType.X
    OP_ADD = mybir.AluOpType.add
    OP_MULT = mybir.AluOpType.mult
    OP_SUB = mybir.AluOpType.subtract

    def r(ap):
        return ap.bitcast(f32r) if USE_F32R else ap

    nc = bacc.Bacc("TRN2", target_bir_lowering=False, debug=False)

    f16 = mybir.dt.float16
    H_d = nc.dram_tensor("h_in", [RX, BL, TX], f16, kind="ExternalInput").ap()
    yT_d = nc.dram_tensor("yt_in", [RX, BL], f16, kind="ExternalInput").ap()
    x0_d = nc.dram_tensor("x0_in", [BL, TX], f32, kind="ExternalInput").ap()
    wdt = f32r if USE_F32R else f32
    wihT_d = nc.dram_tensor("wiht_in", [TX, HID], wdt, kind="ExternalInput").ap()
    whhT_d = nc.dram_tensor("whht_in", [HID, HID], wdt, kind="ExternalInput").ap()
    wxT_d = nc.dram_tensor("wxt_in", [HID, TX], wdt, kind="ExternalInput").ap()
    bh_d = nc.dram_tensor("bh_in", [HID], f32, kind="ExternalInput").ap()
    bx_d = nc.dram_tensor("bx_in", [128, TX], f32, kind="ExternalInput").ap()
    id_d = nc.dram_tensor("id_in", [128, 128], f32, kind="ExternalInput").ap()

    x_out = nc.dram_tensor("x_out", [BL, TX], f32, kind="ExternalOutput").ap()
    h_out = nc.dram_tensor("h_out", [BL, HID], f32, kind="ExternalOutput").ap()
    g_ds = [nc.dram_tensor(f"g_scratch{g}", [128, TX, KCH], mvdt,
                           kind="Internal").ap() for g in range(NG)]

    with ExitStack() as ctx:
        tc = ctx.enter_context(tile.TileContext(nc))

        consts = ctx.enter_context(tc.tile_pool(name="consts", bufs=1))
        abuf = ctx.enter_context(tc.tile_pool(name="abuf", bufs=1))
        mv = ctx.enter_context(tc.tile_pool(name="mv", bufs=2))
        small = ctx.enter_context(tc.tile_pool(name="small", bufs=3))
        hbuf = ctx.enter_context(tc.tile_pool(name="hbuf", bufs=2))
        stage = ctx.enter_context(tc.tile_pool(name="stage", bufs=4))

        # ---- constants ----
        ident = consts.tile([128, 128], f32, tag="ident")
        nc.sync.dma_start(out=ident, in_=id_d)
        yT_sb = consts.tile([RX, BL], f16, tag="yT")
        nc.sync.dma_start(out=yT_sb, in_=yT_d)
        wihT_sb = consts.tile([TX, HID], wdt, tag="wihT")
        nc.sync.dma_start(out=wihT_sb, in_=wihT_d)
        whhT_sb = consts.tile([128, 4, HID], wdt, tag="whhT")
        for k in range(4):
            nc.sync.dma_start(out=whhT_sb[:, k, :], in_=whhT_d[k * 128:(k + 1) * 128, :])
        wxT_sb = consts.tile([128, 4, TX], wdt, tag="wxT")
        for k in range(4):
            nc.sync.dma_start(out=wxT_sb[:, k, :], in_=wxT_d[k * 128:(k + 1) * 128, :])
        bh_sb = consts.tile([128, 4], f32, tag="bh")
        bh_src = bass.AP(tensor=bh_d.tensor, offset=0, ap=[[1, 128], [128, 4]])
        nc.sync.dma_start(out=bh_sb, in_=bh_src)
        bx_sb = consts.tile([128, TX], f32, tag="bx")
        nc.sync.dma_start(out=bx_sb, in_=bx_d)
        bxc = consts.tile([TX, 1], f32, tag="bxc")  # b_x on partitions
        nc.sync.dma_start(out=bxc,
                          in_=bass.AP(tensor=bx_d.tensor, offset=0,
                                      ap=[[1, TX], [1, 1]]))
        x0_sb = consts.tile([128, NG, TX], f32, tag="x0")
        x0_src = bass.AP(tensor=x0_d.tensor, offset=0,
                         ap=[[TX, 128], [128 * TX, NG], [1, TX]])
        nc.sync.dma_start(out=x0_sb, in_=x0_src)

        # ---- phase 1: per-item gram matmuls -> scaled fp16 -> DRAM scratch ----
        # H arrives pre-transposed [r, b, t]: one fully-contiguous DMA.
        htile = abuf.tile([128, BL, TX], f16, tag="hfull")
        HP = BL // 8
        for hc in range(8):
            nc.sync.dma_start(
                out=htile[:, hc * HP:(hc + 1) * HP, :],
                in_=bass.AP(tensor=H_d.tensor, offset=hc * HP * TX,
                            ap=[[BL * TX, 128], [TX, HP], [1, TX]]))
        NS16 = CH // SLOT  # slot-groups per staged write
        with tc.tile_pool(name="gps", bufs=2, space="PSUM") as gps:
            for c0 in range(0, BL, CH):
                st = stage.tile([TX, NS16, SLOT, KCH], mvdt, tag="gstage")
                for s0 in range(NS16):
                    gp = gps.tile([TX, SLOT, TX + 1], f32, tag="gp")
                    for s in range(SLOT):
                        it = c0 + s0 * SLOT + s
                        hb = htile[:, it, :]
                        nc.tensor.matmul(gp[:, s, 0:TX], hb, hb,
                                         start=True, stop=True)
                        nc.tensor.matmul(gp[:, s, TX:TX + 1], hb,
                                         yT_sb[:, it:it + 1],
                                         start=True, stop=True)
                    sv = st[:, s0, :, :]
                    dst_view = bass.AP(tensor=sv.tensor, offset=sv.offset,
                                       ap=[sv.ap[0], [KCH, SLOT], [1, TX + 1]])
                    # scale by -2ss while copying PSUM->SBUF; 1:2 DVE/ACT split
                    if s0 % 3 == 0:
                        nc.vector.tensor_scalar_mul(dst_view, gp, -2.0 * ss)
                    else:
                        nc.scalar.activation(dst_view, gp, Copy, scale=-2.0 * ss)
                stf = st[:, :, :, :]
                src_view = bass.AP(tensor=stf.tensor, offset=stf.offset,
                                   ap=[stf.ap[0], [SLOT * KCH, NS16],
                                       [KCH, SLOT], [1, TX + 1]])
                gd = g_ds[c0 // 128]
                g_dst = bass.AP(tensor=gd.tensor,
                                offset=(c0 % 128) * TX * KCH,
                                ap=[[KCH, TX], [SLOT * TX * KCH, NS16],
                                    [TX * KCH, SLOT], [1, TX + 1]])
                nc.sync.dma_start(out=g_dst, in_=src_view)

        hps = ctx.enter_context(tc.tile_pool(name="hps", bufs=4, space="PSUM"))
        zxps = ctx.enter_context(tc.tile_pool(name="zxps", bufs=1, space="PSUM"))
        xps = ctx.enter_context(tc.tile_pool(name="xps", bufs=1, space="PSUM"))

        # ---- phase 2: read back batch-major; A = I - 2ss*HtH, col64 = -c ----
        A4 = abuf.tile([128, NG, TX, KCH], mvdt, tag="A4")
        for g in range(NG):
            nc.sync.dma_start(
                out=A4[:, g, :, :],
                in_=bass.AP(tensor=g_ds[g].tensor, offset=0,
                            ap=[[TX * KCH, 128], [KCH, TX], [1, KCH]]))
        Af = A4[:, :, :, :]

        def A_view(g):  # [128, 64, 64] fp16: M = -2ss*HtH (no identity)
            return bass.AP(tensor=Af.tensor, offset=Af.offset + g * TX * KCH,
                           ap=[Af.ap[0], [KCH, TX], [1, TX]])

        def negc_view(g):  # [128, 64] strided bf16: -c = -2ss*Hty
            return bass.AP(tensor=Af.tensor,
                           offset=Af.offset + g * TX * KCH + TX,
                           ap=[Af.ap[0], [KCH, TX]])

        # ---- phase 3: iterations ----
        hT_prev = None
        xbf_prev = None   # bf16 [128, NG, TX], current x incl. b_x
        xb_last = None    # fp32 PSUM [128, NG, TX]
        for t in range(iterations):
            # x in fp16 for the matvec (per group, pipelines with transposes)
            xbf = small.tile([128, NG, TX], mvdt, tag="xbf")
            for g in range(NG):
                src = x0_sb[:, g, :] if t == 0 else xb_last[:, g, :]
                nc.vector.tensor_scalar_mul(xbf[:, g, :], src, 1.0)

            # W_hh accumulation first: depends only on hT_prev, overlaps DVE
            hp_tiles = []
            for m in range(4):
                hp = hps.tile([128, BL], f32, tag="hp")
                hp_tiles.append(hp)
                if t > 0:
                    for k in range(4):
                        nc.tensor.matmul(hp,
                                         whhT_sb[:, k, m * 128:(m + 1) * 128],
                                         hT_prev[:, k, :],
                                         start=(k == 0), stop=False)

            # matvec per group + transpose
            zT_ps = zxps.tile([TX, NG, 128], f32, tag="zx")
            for g in range(NG):
                eng = nc.vector
                tm = mv.tile([128, TX * TX], mvdt, tag="tmul")
                xg = xbf[:, g, :]
                x_in = bass.AP(tensor=xg.tensor, offset=xg.offset,
                               ap=[xg.ap[0], [0, TX], [1, TX]])
                eng.tensor_tensor(tm, A_view(g), x_in, op=OP_MULT)
                tmf = tm[:, :]
                w = TX // 2
                while w >= 1:
                    lo = bass.AP(tensor=tmf.tensor, offset=tmf.offset,
                                 ap=[tmf.ap[0], [TX, TX], [1, w]])
                    hi = bass.AP(tensor=tmf.tensor, offset=tmf.offset + w,
                                 ap=[tmf.ap[0], [TX, TX], [1, w]])
                    eng.tensor_tensor(lo, lo, hi, op=OP_ADD)
                    w //= 2
                ssum = bass.AP(tensor=tmf.tensor, offset=tmf.offset,
                               ap=[tmf.ap[0], [TX, TX]])
                zs = small.tile([128, TX], f32, tag="zs")
                nc.vector.tensor_tensor(zs, ssum, negc_view(g), op=OP_SUB)
                xf32 = x0_sb[:, g, :] if t == 0 else xb_last[:, g, :]
                nc.vector.tensor_tensor(zs, zs, xf32, op=OP_ADD)
                nc.tensor.transpose(zT_ps[:, g, :], zs, ident)
            zT_sb = small.tile([TX, NG, 128], wdt, tag="zTsb")
            nc.scalar.activation(zT_sb, zT_ps, Copy)
            zT_flat = bass.AP(tensor=zT_sb.tensor, offset=zT_sb[:, :, :].offset,
                              ap=[zT_sb[:, :, :].ap[0], [1, BL]])

            # W_ih contribution + relu
            hT_new = hbuf.tile([128, 4, HID], wdt, tag="hT")
            for m in range(4):
                nc.tensor.matmul(hp_tiles[m],
                                 wihT_sb[:, m * 128:(m + 1) * 128],
                                 zT_flat,
                                 start=(t == 0), stop=True)
                nc.scalar.activation(hT_new[:, m, :], hp_tiles[m], Relu,
                                     bias=bh_sb[:, m:m + 1])

            # x update: xT = w_x h' (fp32r, N=512), + b_x, transpose to items
            xT_ps = zxps.tile([TX, BL], f32, tag="zx")
            for k in range(4):
                nc.tensor.matmul(xT_ps, wxT_sb[:, k, :], hT_new[:, k, :],
                                 start=(k == 0), stop=(k == 3))
            xT_sb = small.tile([TX, BL], f32, tag="xTsb")
            nc.scalar.activation(xT_sb, xT_ps, Ident, bias=bxc[:, 0:1])
            xb = xps.tile([128, NG, TX], f32, tag="xb")
            for g in range(NG):
                nc.tensor.transpose(xb[:, g, :],
                                    xT_sb[:, g * 128:(g + 1) * 128],
                                    ident[0:TX, 0:TX])
            hT_prev = hT_new
            xb_last = xb

        # ---- outputs ----
        xsb = stage.tile([128, NG, TX], f32, tag="xout")
        nc.vector.tensor_copy(xsb, xb_last)
        x_dst = bass.AP(tensor=x_out.tensor, offset=0,
                        ap=[[TX, 128], [128 * TX, NG], [1, TX]])
        nc.sync.dma_start(out=x_dst, in_=xsb)

        for g in range(NG):
            hsb = stage.tile([128, HID], f32, tag="hsb")
            for m in range(4):
                hp = hps.tile([128, 128], f32, tag="hp")
                nc.tensor.transpose(hp,
                                    hT_prev[:, m, g * 128:(g + 1) * 128].bitcast(f32),
                                    ident)
                nc.vector.tensor_copy(hsb[:, m * 128:(m + 1) * 128], hp)
            nc.sync.dma_start(out=h_out[g * 128:(g + 1) * 128, :], in_=hsb)

    nc.compile()
    return nc


def _get_nc(ss: float, iterations: int):
    key = (round(ss, 12), iterations)
    if key not in _BUILD_CACHE:
        _BUILD_CACHE[key] = _build(ss, iterations)
    return _BUILD_CACHE[key]


def kernel(y, H, x0, W_ih, W_hh, b_ih, b_hh, w_x, b_x, step_size, iterations):
    y = np.asarray(y, np.float32)
    H = np.asarray(H, np.float32)
    x0 = np.asarray(x0, np.float32)
    W_ih = np.asarray(W_ih, np.float32)
    W_hh = np.asarray(W_hh, np.float32)
    b_ih = np.asarray(b_ih, np.float32)
    b_hh = np.asarray(b_hh, np.float32)
    w_x = np.asarray(w_x, np.float32)
    b_x = np.asarray(b_x, np.float32)
    ss = float(np.asarray(step_size).reshape(-1)[0])
    iters = int(iterations)

    if iters == 0:
        return (x0.copy(), np.zeros((y.shape[0], HID), np.float32))

    nc = _get_nc(ss, iters)

    shared = {
        "wiht_in": np.ascontiguousarray(W_ih.T),
        "whht_in": np.ascontiguousarray(W_hh.T),
        "wxt_in": np.ascontiguousarray(w_x.T),
        "bh_in": np.ascontiguousarray(b_ih + b_hh),
        "bx_in": np.ascontiguousarray(np.tile(b_x[None, :], (128, 1))),
        "id_in": np.eye(128, dtype=np.float32),
    }
    in_maps = []
    for c in range(NCORES):
        sl = slice(c * BL, (c + 1) * BL)
        m = dict(shared)
        m["h_in"] = np.ascontiguousarray(H[sl].transpose(1, 0, 2)).astype(np.float16)
        m["yt_in"] = np.ascontiguousarray(y[sl].T).astype(np.float16)
        m["x0_in"] = np.ascontiguousarray(x0[sl, :, 0])
        in_maps.append(m)

    from concourse.bass_utils import run_bass_kernel_spmd
    res = run_bass_kernel_spmd(nc, in_maps, core_ids=list(range(NCORES)))
    x = np.concatenate([res.results[c]["x_out"] for c in range(NCORES)], axis=0)
    h = np.concatenate([res.results[c]["h_out"] for c in range(NCORES)], axis=0)
    return (x[:, :, None].astype(np.float32), h.astype(np.float32))


# revision 29
# speedup vs baseline: 201.5109x; 201.5109x over previous
"""Trainium2 Bass kernel for nn_DetModel_77738908057822.

Computation (per batch item b, `iterations` steps):
    HtH_b = H_b^T H_b   (64x64), Hty_b = H_b^T y_b  (64)
    z = x + 2*ss*(Hty - HtH x) = A x + c,  A = I - 2ss*HtH, c = 2ss*Hty
    h' = relu(z @ W_ih^T + h @ W_hh^T + b_ih + b_hh)
    x' = h' @ w_x^T + b_x
Returns (x_N [B,64,1], h_N [B,512]).

Sharding: pure data parallel over batch across 8 cores (512 items/core).

Per-core plan (512 items = 4 groups of 128):
  Phase 1 (PE): host packs Hy = [H_b | y_b] as fp16 [r, b, 65]; per-item
     gram matmul lhsT=H_b rhs=Hy_b -> PSUM [64, 65] = [HtH | Hty], two
     items stacked per PSUM bank via tile_position col-tiling.  Scale by
     -2ss while copying PSUM->SBUF (fp16 out), batched DMA to DRAM
     scratch [item, j, 66] (col 64 = -c = -2ss*Hty, col 65 pad).
  Phase 2: DMA scratch back contiguous -> M [128 items-on-partitions,
     g, j, 66] fp16 = -2ss*HtH (compensated form: no +I, so fp16
     rounding never touches the dominant identity term).
  Phase 3 per iteration (z = x + M x + c computed as fp32 x plus small
  fp16 correction; weights/zT/hT typed float32r so the PE streams
  1 cycle/column):
     PE:  hp[m] += sum_k W_hhT_km^T hT_k      (overlaps the DVE matvec)
     DVE per group: t = M_view * x_bcast (fp16 2x); 6 in-place tree
          folds over k (fp16 2x); z = (fold - (-c)) + x_fp32
     PE:  zT = transpose(z) -> per-group ACT copy -> zT_sb (f32r)
          hp[m] += W_ihT_m^T zT
     DVE/ACT: hT'[m] = relu(hp[m] + bh[m])   (2 on DVE, 2 on ACT)
     PE:  xT = sum_k wxT_k^T hT'_k -> ACT Identity+b_x -> xT_sb
     PE:  xb[g] = transpose(xT_g)  (PSUM, items on partitions, incl b_x)
     DVE: xbf[g] = fp16(xb[g])  (feeds next iteration's matvec)
  Final x = xb; h via 16 PE transposes.

Fixed phase is DMA-bound (~17 MB: fp16 H load + fp16 scratch roundtrip
for the j->item partition transpose); iterations are DVE-bound (the
per-batch 64x64 matvec has no shared-weight structure for the PE).
"""

import sys
import numpy as np

for _p in ("/opt/trn_rl_repo",):
    if _p not in sys.path:
        sys.path.insert(0, _p)

B, RX, TX, HID = 4096, 128, 64, 512
NCORES = 8
BL = B // NCORES          # 512 items per core
NG = BL // 128            # 4 groups of 128 items
KCH = TX + 2              # 66: [HtH(64) | Hty(1) | pad(1)] per item row
CH = 64                   # items per staged scratch write
SLOT = 4                  # items per gram psum tile

USE_F32R = True           # fp32r (1 cyc/col) for the big matmuls
USE_BF16_MV = True        # bf16 A + bf16 tree-fold matvec

_BUILD_CACHE = {}


def _build(ss: float, iterations: int):
    from contextlib import ExitStack

    import concourse.bacc as bacc
    import concourse.bass as bass
    import concourse.tile as tile
    from concourse import mybir

    f32 = mybir.dt.float32
    f32r = mybir.dt.float32r
    mvdt = mybir.dt.float16 if USE_BF16_MV else f32
    Relu = mybir.ActivationFunctionType.Relu
    Copy = mybir.ActivationFunctionType.Copy
    Ident = mybir.ActivationFunctionType.Identity
    AXX = mybir.AxisListType.X
    OP_ADD = mybir.AluOpType.add
    OP_MULT = mybir.AluOpType.mult
    OP_SUB = mybir.AluOpType.subtract
    OP_MAX = mybir.AluOpType.max

    def r(ap):
        return ap.bitcast(f32r) if USE_F32R else ap

    nc = bacc.Bacc("TRN2", target_bir_lowering=False, debug=False)

    f16 = mybir.dt.float16
    # h_in is host-packed [r, b, 65]: cols 0:64 = H[b,r,:], col 64 = y[b,r]
    H_d = nc.dram_tensor("h_in", [RX, BL, TX + 1], f16, kind="ExternalInput").ap()
    x0_d = nc.dram_tensor("x0_in", [BL, TX], f32, kind="ExternalInput").ap()
    wdt = f32r if USE_F32R else f32
    wihT_d = nc.dram_tensor("wiht_in", [TX, HID], wdt, kind="ExternalInput").ap()
    whhT_d = nc.dram_tensor("whht_in", [HID, HID], wdt, kind="ExternalInput").ap()
    wxT_d = nc.dram_tensor("wxt_in", [HID, TX], wdt, kind="ExternalInput").ap()
    bh_d = nc.dram_tensor("bh_in", [HID], f32, kind="ExternalInput").ap()
    bx_d = nc.dram_tensor("bx_in", [128, TX], f32, kind="ExternalInput").ap()
    id_d = nc.dram_tensor("id_in", [128, 128], f32, kind="ExternalInput").ap()

    x_out = nc.dram_tensor("x_out", [BL, TX], f32, kind="ExternalOutput").ap()
    h_out = nc.dram_tensor("h_out", [BL, HID], f32, kind="ExternalOutput").ap()
    g_ds = [nc.dram_tensor(f"g_scratch{g}", [128, TX, KCH], mvdt,
                           kind="Internal").ap() for g in range(NG)]

    with ExitStack() as ctx:
        tc = ctx.enter_context(tile.TileContext(nc))

        consts = ctx.enter_context(tc.tile_pool(name="consts", bufs=1))
        abuf = ctx.enter_context(tc.tile_pool(name="abuf", bufs=1))
        mv = ctx.enter_context(tc.tile_pool(name="mv", bufs=2))
        small = ctx.enter_context(tc.tile_pool(name="small", bufs=3))
        hbuf = ctx.enter_context(tc.tile_pool(name="hbuf", bufs=2))
        stage = ctx.enter_context(tc.tile_pool(name="stage", bufs=4))

        # ---- constants ----
        ident = consts.tile([128, 128], f32, tag="ident")
        nc.sync.dma_start(out=ident, in_=id_d)
        wihT_sb = consts.tile([TX, HID], wdt, tag="wihT")
        nc.sync.dma_start(out=wihT_sb, in_=wihT_d)
        whhT_sb = consts.tile([128, 4, HID], wdt, tag="whhT")
        for k in range(4):
            nc.sync.dma_start(out=whhT_sb[:, k, :], in_=whhT_d[k * 128:(k + 1) * 128, :])
        wxT_sb = consts.tile([128, 4, TX], wdt, tag="wxT")
        for k in range(4):
            nc.sync.dma_start(out=wxT_sb[:, k, :], in_=wxT_d[k * 128:(k + 1) * 128, :])
        bh_sb = consts.tile([128, 4], f32, tag="bh")
        bh_src = bass.AP(tensor=bh_d.tensor, offset=0, ap=[[1, 128], [128, 4]])
        nc.sync.dma_start(out=bh_sb, in_=bh_src)
        bx_sb = consts.tile([128, TX], f32, tag="bx")
        nc.sync.dma_start(out=bx_sb, in_=bx_d)
        bxc = consts.tile([TX, 1], f32, tag="bxc")  # b_x on partitions
        nc.sync.dma_start(out=bxc,
                          in_=bass.AP(tensor=bx_d.tensor, offset=0,
                                      ap=[[1, TX], [1, 1]]))
        x0_sb = consts.tile([128, NG, TX], f32, tag="x0")
        x0_src = bass.AP(tensor=x0_d.tensor, offset=0,
                         ap=[[TX, 128], [128 * TX, NG], [1, TX]])
        nc.sync.dma_start(out=x0_sb, in_=x0_src)

        # ---- phase 1: per-item gram matmuls -> scaled fp16 -> DRAM scratch ----
        # H arrives pre-transposed [r, b, t]: one fully-contiguous DMA.
        htile = abuf.tile([128, BL, TX + 1], f16, tag="hfull")
        HP = BL // 8
        for hc in range(8):
            nc.sync.dma_start(
                out=htile[:, hc * HP:(hc + 1) * HP, :],
                in_=bass.AP(tensor=H_d.tensor, offset=hc * HP * (TX + 1),
                            ap=[[BL * (TX + 1), 128], [TX + 1, HP],
                                [1, TX + 1]]))
        # item PAIRS stacked on partitions: even item -> rows 0:64 (PE cols
        # 0:63), odd item -> rows 64:128 via tile_position col-tiling.
        NT8 = CH // (2 * SLOT)  # gp tiles per staged chunk (8 items each)
        ITEM = TX * KCH
        with tc.tile_pool(name="gps", bufs=2, space="PSUM") as gps:
            for c0 in range(0, BL, CH):
                st = stage.tile([128, NT8, SLOT, KCH], mvdt, tag="gstage")
                for t8 in range(NT8):
                    gp = gps.tile([128, 512], f32, tag="gp")
                    for s in range(SLOT):
                        for half in range(2):
                            it = c0 + t8 * 2 * SLOT + 2 * s + half
                            hb = htile[:, it, 0:TX]
                            o0 = s * (TX + 1)
                            out = gp[half * TX:(half + 1) * TX,
                                     o0:o0 + TX + 1]
                            tp = (0, TX) if half else None
                            nc.tensor.matmul(out, hb, htile[:, it, :],
                                             start=True, stop=True,
                                             tile_position=tp)
                    sv = st[:, t8, :, :]
                    dst_view = bass.AP(tensor=sv.tensor, offset=sv.offset,
                                       ap=[sv.ap[0], [KCH, SLOT], [1, TX + 1]])
                    gpf = gp[:, :]
                    gp_view = bass.AP(tensor=gpf.tensor, offset=gpf.offset,
                                      ap=[gpf.ap[0], [TX + 1, SLOT], [1, TX + 1]])
                    if t8 % 3 == 0:
                        nc.vector.tensor_scalar_mul(dst_view, gp_view, -2.0 * ss)
                    else:
                        nc.scalar.activation(dst_view, gp_view, Copy, scale=-2.0 * ss)
                stf = st[:, :, :, :]
                gd = g_ds[c0 // 128]
                for half in range(2):
                    src_view = bass.AP(
                        tensor=stf.tensor,
                        offset=stf.offset + half * TX * SLOT * KCH * NT8,
                        ap=[[SLOT * KCH * NT8, TX], [SLOT * KCH, NT8],
                            [KCH, SLOT], [1, TX + 1]])
                    g_dst = bass.AP(
                        tensor=gd.tensor,
                        offset=(c0 % 128) * ITEM + half * ITEM,
                        ap=[[KCH, TX], [2 * SLOT * ITEM, NT8],
                            [2 * ITEM, SLOT], [1, TX + 1]])
                    nc.sync.dma_start(out=g_dst, in_=src_view)

        hps = ctx.enter_context(tc.tile_pool(name="hps", bufs=4, space="PSUM"))
        zxps = ctx.enter_context(tc.tile_pool(name="zxps", bufs=1, space="PSUM"))
        xps = ctx.enter_context(tc.tile_pool(name="xps", bufs=1, space="PSUM"))

        # ---- phase 2: read back batch-major; A = I - 2ss*HtH, col64 = -c ----
        A4 = abuf.tile([128, NG, TX, KCH], mvdt, tag="A4")
        for g in range(NG):
            nc.sync.dma_start(
                out=A4[:, g, :, :],
                in_=bass.AP(tensor=g_ds[g].tensor, offset=0,
                            ap=[[TX * KCH, 128], [KCH, TX], [1, KCH]]))
        Af = A4[:, :, :, :]

        def A_view(g):  # [128, 64, 64] fp16: M = -2ss*HtH (no identity)
            return bass.AP(tensor=Af.tensor, offset=Af.offset + g * TX * KCH,
                           ap=[Af.ap[0], [KCH, TX], [1, TX]])

        def negc_view(g):  # [128, 64] strided bf16: -c = -2ss*Hty
            return bass.AP(tensor=Af.tensor,
                           offset=Af.offset + g * TX * KCH + TX,
                           ap=[Af.ap[0], [KCH, TX]])

        # ---- phase 3: iterations ----
        hT_prev = None
        xbf_prev = None   # bf16 [128, NG, TX], current x incl. b_x
        xb_last = None    # fp32 PSUM [128, NG, TX]
        for t in range(iterations):
            # x in fp16 for the matvec (per group, pipelines with transposes)
            xbf = small.tile([128, NG, TX], mvdt, tag="xbf")
            for g in range(NG):
                src = x0_sb[:, g, :] if t == 0 else xb_last[:, g, :]
                nc.vector.tensor_scalar_mul(xbf[:, g, :], src, 1.0)

            # W_hh accumulation first: depends only on hT_prev, overlaps DVE
            hp_tiles = []
            for m in range(4):
                hp = hps.tile([128, BL], f32, tag="hp")
                hp_tiles.append(hp)
                if t > 0:
                    for k in range(4):
                        nc.tensor.matmul(hp,
                                         whhT_sb[:, k, m * 128:(m + 1) * 128],
                                         hT_prev[:, k, :],
                                         start=(k == 0), stop=False)

            # matvec per group + transpose
            zT_ps = zxps.tile([TX, NG, 128], f32, tag="zx")
            for g in range(NG):
                eng = nc.vector
                tm = mv.tile([128, TX * TX], mvdt, tag="tmul")
                xg = xbf[:, g, :]
                x_in = bass.AP(tensor=xg.tensor, offset=xg.offset,
                               ap=[xg.ap[0], [0, TX], [1, TX]])
                eng.tensor_tensor(tm, A_view(g), x_in, op=OP_MULT)
                tmf = tm[:, :]
                w = TX // 2
                while w >= 1:
                    lo = bass.AP(tensor=tmf.tensor, offset=tmf.offset,
                                 ap=[tmf.ap[0], [TX, TX], [1, w]])
                    hi = bass.AP(tensor=tmf.tensor, offset=tmf.offset + w,
                                 ap=[tmf.ap[0], [TX, TX], [1, w]])
                    eng.tensor_tensor(lo, lo, hi, op=OP_ADD)
                    w //= 2
                ssum = bass.AP(tensor=tmf.tensor, offset=tmf.offset,
                               ap=[tmf.ap[0], [TX, TX]])
                zs = small.tile([128, TX], f32, tag="zs")
                nc.vector.tensor_tensor(zs, ssum, negc_view(g), op=OP_SUB)
                xf32 = x0_sb[:, g, :] if t == 0 else xb_last[:, g, :]
                nc.vector.tensor_tensor(zs, zs, xf32, op=OP_ADD)
                nc.tensor.transpose(zT_ps[:, g, :], zs, ident)
                if g == 0:
                    zT_sb = small.tile([TX, NG, 128], wdt, tag="zTsb")
                nc.scalar.activation(zT_sb[:, g, :], zT_ps[:, g, :], Copy)
            zT_flat = bass.AP(tensor=zT_sb.tensor, offset=zT_sb[:, :, :].offset,
                              ap=[zT_sb[:, :, :].ap[0], [1, BL]])

            # W_ih contribution + relu
            hT_new = hbuf.tile([128, 4, HID], wdt, tag="hT")
            for m in range(4):
                nc.tensor.matmul(hp_tiles[m],
                                 wihT_sb[:, m * 128:(m + 1) * 128],
                                 zT_flat,
                                 start=(t == 0), stop=True)
                if m < 2:
                    nc.vector.tensor_scalar(hT_new[:, m, :], hp_tiles[m],
                                            bh_sb[:, m:m + 1], 0.0,
                                            op0=OP_ADD, op1=OP_MAX)
                else:
                    nc.scalar.activation(hT_new[:, m, :], hp_tiles[m], Relu,
                                         bias=bh_sb[:, m:m + 1])

            # x update: xT = w_x h' (fp32r, N=512), + b_x, transpose to items
            xT_ps = zxps.tile([TX, BL], f32, tag="zx")
            for k in range(4):
                nc.tensor.matmul(xT_ps, wxT_sb[:, k, :], hT_new[:, k, :],
                                 start=(k == 0), stop=(k == 3))
            xT_sb = small.tile([TX, BL], f32, tag="xTsb")
            nc.scalar.activation(xT_sb, xT_ps, Ident, bias=bxc[:, 0:1])
            xb = xps.tile([128, NG, TX], f32, tag="xb")
            for g in range(NG):
                nc.tensor.transpose(xb[:, g, :],
                                    xT_sb[:, g * 128:(g + 1) * 128],
                                    ident[0:TX, 0:TX])
            hT_prev = hT_new
            xb_last = xb

        # ---- outputs ----
        xsb = stage.tile([128, NG, TX], f32, tag="xout")
        nc.vector.tensor_copy(xsb, xb_last)
        x_dst = bass.AP(tensor=x_out.tensor, offset=0,
                        ap=[[TX, 128], [128 * TX, NG], [1, TX]])
        nc.sync.dma_start(out=x_dst, in_=xsb)

        for g in range(NG):
            hsb = stage.tile([128, HID], f32, tag="hsb")
            for m in range(4):
                hp = hps.tile([128, 128], f32, tag="hp")
                nc.tensor.transpose(hp,
                                    hT_prev[:, m, g * 128:(g + 1) * 128].bitcast(f32),
                                    ident)
                nc.vector.tensor_copy(hsb[:, m * 128:(m + 1) * 128], hp)
            nc.sync.dma_start(out=h_out[g * 128:(g + 1) * 128, :], in_=hsb)

    nc.compile()
    return nc


def _get_nc(ss: float, iterations: int):
    key = (round(ss, 12), iterations)
    if key not in _BUILD_CACHE:
        _BUILD_CACHE[key] = _build(ss, iterations)
    return _BUILD_CACHE[key]


def kernel(y, H, x0, W_ih, W_hh, b_ih, b_hh, w_x, b_x, step_size, iterations):
    y = np.asarray(y, np.float32)
    H = np.asarray(H, np.float32)
    x0 = np.asarray(x0, np.float32)
    W_ih = np.asarray(W_ih, np.float32)
    W_hh = np.asarray(W_hh, np.float32)
    b_ih = np.asarray(b_ih, np.float32)
    b_hh = np.asarray(b_hh, np.float32)
    w_x = np.asarray(w_x, np.float32)
    b_x = np.asarray(b_x, np.float32)
    ss = float(np.asarray(step_size).reshape(-1)[0])
    iters = int(iterations)

    if iters == 0:
        return (x0.copy(), np.zeros((y.shape[0], HID), np.float32))

    nc = _get_nc(ss, iters)

    shared = {
        "wiht_in": np.ascontiguousarray(W_ih.T),
        "whht_in": np.ascontiguousarray(W_hh.T),
        "wxt_in": np.ascontiguousarray(w_x.T),
        "bh_in": np.ascontiguousarray(b_ih + b_hh),
        "bx_in": np.ascontiguousarray(np.tile(b_x[None, :], (128, 1))),
        "id_in": np.eye(128, dtype=np.float32),
    }
    in_maps = []
    for c in range(NCORES):
        sl = slice(c * BL, (c + 1) * BL)
        m = dict(shared)
        hy = np.concatenate([H[sl].transpose(1, 0, 2), y[sl].T[:, :, None]],
                            axis=2)
        m["h_in"] = np.ascontiguousarray(hy).astype(np.float16)
        m["x0_in"] = np.ascontiguousarray(x0[sl, :, 0])
        in_maps.append(m)

    from concourse.bass_utils import run_bass_kernel_spmd
    res = run_bass_kernel_spmd(nc, in_maps, core_ids=list(range(NCORES)))
    x = np.concatenate([res.results[c]["x_out"] for c in range(NCORES)], axis=0)
    h = np.concatenate([res.results[c]["h_out"] for c in range(NCORES)], axis=0)
    return (x[:, :, None].astype(np.float32), h.astype(np.float32))
